# revision 23
# baseline (speedup 1.0000x reference)
"""Trainium2 8-core kernel for the LaneGCN-style A2A message-passing block.

Strategy (v2, memory-regime):
  - Host: sort edges by destination (hi); partition the 157 global 128-node
    windows across 8 cores with greedy edge balancing; per core, order its 20
    window slots by descending edge count so the SPMD-shared per-slot padding
    (max across cores) stays tight. Pre-gather per edge: ctx row, dctr (+ones
    row so dist_b1 folds into the matmul), and BOTH one-hot incidence layouts
    (edge-major for scatter, node-major for gather) as bf16 DMA payloads.
  - Device per 128-edge tile (all operands bf16, PSUM fp32):
      h   = relu(W1a @ dctr3)                    (per super, ch-major, N=512)
      dp  = h_tile' @ W2ᵀ                        (edge-major)
      d   = relu(GN(dp))      via bn_stats/aggr + fused ACT scale/bias/relu
      d_cm = DMA-XBAR transpose of d             (no PE transpose, no copy)
      u   = ctxg@Cᵀ + onehotᵀ@qb_tab + d@Aᵀ      (3 accumulated MMs)
      c   = relu(GN(u))       same fused pattern
      s[win] += cᵀ @ onehot                      (scatter matmul into PSUM)
    GN row math (sqrt/recip/-m*r) is pair-batched across 2 tiles to amortize
    the fixed per-op engine overhead.
  - Node phases: qb_tab = (relu(GN(agts@q_wᵀ)))@Bᵀ per window (phase 1);
    epilogue GN->lin->GN->residual->relu per window, emitted inline at each
    window's last scatter so it overlaps the edge pipeline (phase 3).
"""

import sys

import numpy as np

if "/opt/trn_rl_repo" not in sys.path:
    sys.path.insert(0, "/opt/trn_rl_repo")

import ml_dtypes

import concourse.bass as bass
import concourse.mybir as mybir
import concourse.tile as tile
from concourse.bass_utils import run_bass_kernel_spmd

N_NODES = 20000
D = 128
NC = 8
NWIN = 20            # window slots per core (157 global windows over 8 cores)
NPAD = NWIN * 128
GWIN = (N_NODES + 127) // 128
F32 = mybir.dt.float32
BF16 = mybir.dt.bfloat16
NPBF = ml_dtypes.bfloat16


def _apply_drain_patch():
    """This neuronxcc build rejects >2 sem waits on the Tile tail drain
    ("Too many sync wait commands"); split them into single-sem SP waits."""
    from concourse.vector_clock import ScopedClock

    if getattr(tile.TileContext, "_drain_patched", False):
        return

    def _patched(self, tick_clock, wait_clock):
        nc = self.nc
        probe = nc.sync.nop(nofuse=True, hint="drain_wait_probe")
        wait_clock.add_sem_waits(
            probe.ins, ScopedClock({None: tick_clock.global_clock})
        )
        si = probe.ins.sync_info
        waits = list(si.on_wait) if si and si.on_wait else []
        sem_by_id = {h.num: h for h in self.sems.allocated().values()}
        if len(waits) > 2:
            si.on_wait.clear()
            for w in waits:
                h = sem_by_id[w.id]
                nc.sync.wait_ge(h, w.wait_value)
        nc.sync.drain()
        nc.all_engine_barrier()
        popped = nc._tile_sem_poison_stack.pop()
        assert popped is self._sem_poison
        nc.clear_and_free_semaphores(list(self.sems.allocated().values()))
        nc.all_engine_barrier()

    tile.TileContext._drain_and_barrier = _patched
    tile.TileContext._drain_patched = True


def _split_excess_waits(nc, max_waits=1):
    """walrus here rejects instructions with >2 sem-wait commands; hoist the
    excess onto single-wait NoOps inserted just before (same engine)."""
    n = 0
    for f in nc.m.functions:
        for bb in f.blocks:
            out = []
            changed = False
            for ins in bb.instructions:
                si = ins.sync_info
                waits = list(si.on_wait) if si and si.on_wait else []
                if len(waits) > max_waits:
                    keep = waits[-max_waits:]
                    for w in waits[:-max_waits]:
                        nop = mybir.InstNoOp(
                            name=f"I-waitfix-{n}", engine=ins.engine
                        )
                        n += 1
                        nop.sync_info = mybir.SyncInfo(
                            on_wait=[w], on_update=[]
                        )
                        out.append(nop)
                    ins.sync_info = mybir.SyncInfo(
                        on_wait=keep,
                        on_update=list(si.on_update) if si.on_update else [],
                    )
                    changed = True
                out.append(ins)
            if changed:
                bb.instructions = out


def _prep(inputs):
    """Sort/balance/pad edges; build per-core bf16 device arrays."""
    f = lambda k: np.asarray(inputs[k], dtype=np.float32)
    agts = f("agts")
    ctx = f("ctx")
    agt_ctrs = f("agt_ctrs")
    ctx_ctrs = f("ctx_ctrs")
    hi = np.asarray(inputs["hi"], dtype=np.int64)
    wi = np.asarray(inputs["wi"], dtype=np.int64)

    for g, b in (("dist_g", "dist_beta"), ("q_g", "q_beta"),
                 ("ctx_g", "ctx_beta"), ("norm_g", "norm_beta"),
                 ("lin_g", "lin_beta")):
        assert np.allclose(np.asarray(inputs[g]), 1.0), f"{g} != 1 unsupported"
        assert np.allclose(np.asarray(inputs[b]), 0.0), f"{b} != 0 unsupported"

    order = np.argsort(hi, kind="stable")
    hi_s = hi[order]
    wi_s = wi[order]

    starts = np.searchsorted(hi_s, np.arange(GWIN) * 128)
    ends = np.searchsorted(
        hi_s, np.minimum((np.arange(GWIN) + 1) * 128, N_NODES))
    cnt = ends - starts

    # greedy balance: biggest windows first to the least-loaded core
    core_load = np.zeros(NC, np.int64)
    core_wins = [[] for _ in range(NC)]
    for w in np.argsort(-cnt, kind="stable"):
        cands = [c for c in range(NC) if len(core_wins[c]) < NWIN]
        c = min(cands, key=lambda c: core_load[c])
        core_wins[c].append(w)
        core_load[c] += cnt[w]

    slot_win = np.full((NC, NWIN), -1, np.int64)
    for c in range(NC):
        ws = sorted(core_wins[c], key=lambda w: -cnt[w])
        slot_win[c, :len(ws)] = ws
    slot_cnt = np.where(slot_win >= 0, cnt[np.maximum(slot_win, 0)], 0)

    wk = ((slot_cnt.max(axis=0) + 127) // 128) * 128
    wk = np.maximum(wk, 128)
    e_pad = int(wk.sum())
    extra = (-e_pad) % 512
    wk[NWIN - 1] += extra
    e_pad += extra
    woff = np.concatenate([[0], np.cumsum(wk)]).astype(np.int64)
    n_tiles = e_pad // 128

    tile_slot = np.empty(n_tiles, np.int64)
    for k in range(NWIN):
        tile_slot[woff[k] // 128: woff[k + 1] // 128] = k
    first_tile = (woff[:-1] // 128).astype(np.int64)
    last_tile = (woff[1:] // 128 - 1).astype(np.int64)

    bf = lambda a: np.ascontiguousarray(a).astype(NPBF)
    w1 = f("dist_w1")                                          # [D, 2]
    b1 = f("dist_b1")                                          # [D]
    cw1 = f("ctx_w1")                                          # [D, 3D]
    w1a = np.zeros((128, 128), np.float32)
    w1a[0:2] = w1.T
    w1a[2] = b1
    shared_cols = bf(np.concatenate(
        [w1a, f("dist_w2").T, cw1[:, :D].T, cw1[:, D:2 * D].T,
         cw1[:, 2 * D:].T, f("q_w").T, f("ctx_w2").T, f("agt_w").T,
         f("lin_w").T], axis=1))                               # [128, 1152]

    per_core = []
    for c in range(NC):
        dctr3 = np.zeros((3, e_pad), np.float32)
        ctxg = np.zeros((e_pad, D), np.float32)
        seg = np.full(e_pad, -1, np.int64)
        nodes = np.full(NWIN * 128, -1, np.int64)
        for k in range(NWIN):
            w = int(slot_win[c, k])
            if w < 0:
                continue
            n = int(cnt[w])
            s0 = int(starts[w])
            d0 = int(woff[k])
            idx = slice(s0, s0 + n)
            dctr3[0, d0:d0 + n] = agt_ctrs[hi_s[idx], 0] - ctx_ctrs[wi_s[idx], 0]
            dctr3[1, d0:d0 + n] = agt_ctrs[hi_s[idx], 1] - ctx_ctrs[wi_s[idx], 1]
            dctr3[2, d0:d0 + n] = 1.0
            ctxg[d0:d0 + n] = ctx[wi_s[idx]]
            seg[d0:d0 + n] = hi_s[idx] - 128 * w
            base = 128 * w
            nv = min(128, N_NODES - base)
            nodes[k * 128:k * 128 + nv] = np.arange(base, base + nv)

        # per-tile seg values, edge-major [128, n_tiles]: column t holds the
        # window-local destination offset of each edge in tile t (-1 pad)
        seg_pm = np.ascontiguousarray(
            seg.reshape(n_tiles, 128).T).astype(np.float32)
        valid = nodes >= 0
        ag = agts[np.maximum(nodes, 0)] * valid[:, None]       # [NPAD, D]
        # head blob: [w1aT pad | weights 8x128 | agts_cm | agts_nm]
        head = np.concatenate(
            [shared_cols, np.ascontiguousarray(ag.T).astype(NPBF),
             np.ascontiguousarray(
                 ag.reshape(NWIN, 128, D).transpose(1, 0, 2)
                 .reshape(128, NWIN * D)).astype(NPBF)], axis=1)
        per_core.append(dict(
            dctr=np.ascontiguousarray(dctr3).astype(NPBF),
            big=np.ascontiguousarray(ctxg.T).astype(NPBF),     # [D, e_pad]
            head=np.ascontiguousarray(head),
            seg_pm=seg_pm,
        ))

    meta = dict(e_pad=e_pad, n_tiles=n_tiles, tile_slot=tile_slot,
                first_tile=first_tile, last_tile=last_tile, slot_win=slot_win)
    return per_core, {}, meta


def _build(meta):
    nc = bass.Bass()
    e_pad = meta["e_pad"]
    n_tiles = meta["n_tiles"]
    tile_slot = meta["tile_slot"]
    first_tile = meta["first_tile"]
    last_tile = meta["last_tile"]
    n_super = e_pad // 512
    AF = mybir.ActivationFunctionType
    OP = mybir.AluOpType

    din = {}
    for name, shape, dt in [
        ("dctr", [3, e_pad], BF16), ("big", [D, e_pad], BF16),
        ("head", [128, 1152 + 2 * NPAD], BF16),
        ("seg_pm", [128, n_tiles], F32),
    ]:
        din[name] = nc.dram_tensor(name, shape, dt, kind="ExternalInput")
    out_d = nc.dram_tensor("out", [NPAD, D], F32, kind="ExternalOutput")

    with tile.TileContext(nc) as tc:
        with (
            tc.tile_pool(name="consts", bufs=1) as consts,
            tc.tile_pool(name="io", bufs=3) as io,
            tc.tile_pool(name="wkp", bufs=4) as wkp,
            tc.tile_pool(name="ep", bufs=2) as ep,
            tc.tile_pool(name="sm", bufs=4) as sm,
            tc.tile_pool(name="smp", bufs=3) as smp,
            tc.tile_pool(name="ph", bufs=1, space="PSUM") as ph,
            tc.tile_pool(name="pdp", bufs=3, space="PSUM") as pdp,
            tc.tile_pool(name="pu", bufs=3, space="PSUM") as pu,
            tc.tile_pool(name="pwin", bufs=1, space="PSUM") as pwin,
        ):
            cs = {}
            names = ("w1aT", "w2T", "AT", "BT", "CT", "qwT", "xw2T",
                     "awT", "lwT")
            for idx, name in enumerate(names):
                shape = [3, D] if name == "w1aT" else [D, D]
                t = consts.tile(shape, BF16, tag=f"c_{name}", name=f"c_{name}")
                nc.sync.dma_start(
                    out=t[:],
                    in_=din["head"][0:shape[0], idx * 128:idx * 128 + 128])
                cs[name] = t
            agts_cm = consts.tile([D, NPAD], BF16, tag="c_agcm")
            nc.sync.dma_start(out=agts_cm[:],
                              in_=din["head"][:, 1152:1152 + NPAD])
            agts_nm = consts.tile([128, NWIN * D], BF16, tag="c_agnm")
            nc.sync.dma_start(out=agts_nm[:],
                              in_=din["head"][:, 1152 + NPAD:1152 + 2 * NPAD])
            seg_sb = consts.tile([128, n_tiles], F32, tag="c_seg")
            nc.sync.dma_start(out=seg_sb[:], in_=din["seg_pm"][:])
            qb_tab = consts.tile([128, NWIN, 128], BF16, tag="c_qbtab")
            s_tab = consts.tile([128, NWIN, 128], BF16, tag="c_stab")
            eps_t = consts.tile([128, 1], F32, tag="c_eps")
            nc.vector.memset(eps_t[:], 1e-5)
            iota_t = consts.tile([128, 128], F32, tag="c_iota")
            nc.gpsimd.iota(iota_t[:], pattern=[[1, 128]], base=0,
                           channel_multiplier=0,
                           allow_small_or_imprecise_dtypes=True)

            def gn_small(ps, tagp):
                """Unbatched GN row stats for node phases: returns (r, b)."""
                st = smp.tile([128, 6], F32, tag=f"{tagp}st", name=f"{tagp}st")
                nc.vector.bn_stats(out=st[:], in_=ps[:])
                mv = smp.tile([128, 2], F32, tag=f"{tagp}mv", name=f"{tagp}mv")
                nc.vector.bn_aggr(out=mv[:], in_=st[:])
                sd = smp.tile([128, 1], F32, tag=f"{tagp}sd", name=f"{tagp}sd")
                nc.scalar.activation(out=sd[:], in_=mv[:, 1:2], func=AF.Sqrt,
                                     bias=eps_t[:], scale=1.0)
                r = smp.tile([128, 1], F32, tag=f"{tagp}r", name=f"{tagp}r")
                nc.vector.reciprocal(out=r[:], in_=sd[:])
                b = smp.tile([128, 1], F32, tag=f"{tagp}b", name=f"{tagp}b")
                nc.vector.tensor_scalar(
                    out=b[:], in0=mv[:, 0:1], scalar1=r[:], scalar2=-1.0,
                    op0=OP.mult, op1=OP.mult,
                )
                return r, b

            # ---- phase 1: qb table (q -> GN -> relu -> @B^T per window) ----
            for w in range(NWIN):
                qp = pdp.tile([128, 128], F32, tag="dp", name=f"qp{w}")
                nc.tensor.matmul(out=qp[:],
                                 lhsT=agts_cm[:, w * 128:(w + 1) * 128],
                                 rhs=cs["qwT"][:], start=True, stop=True)
                r1, b1 = gn_small(qp, "p1")
                q_sb = ep.tile([128, 128], BF16, tag="qsb", name=f"qsb{w}")
                nc.scalar.activation(out=q_sb[:], in_=qp[:], func=AF.Relu,
                                     bias=b1[:], scale=r1[:])
                q_cm = ep.tile([128, 128], BF16, tag="qcm", name=f"qcm{w}")
                nc.scalar.dma_start(out=q_cm[:], in_=q_sb[:], transpose=True)
                qbp = pu.tile([128, 128], F32, tag="u", name=f"qbp{w}")
                nc.tensor.matmul(out=qbp[:], lhsT=q_cm[:], rhs=cs["BT"][:],
                                 start=True, stop=True)
                nc.vector.tensor_copy(out=qb_tab[:, w, :], in_=qbp[:])

            # ---- phase 3, two stages so the a1 transpose latency is hidden
            # behind a few supers of edge work before the tail runs ----
            ep_pending = []

            def epilogue_head(k):
                ap = pdp.tile([128, 128], F32, tag="dp", name=f"ap{k}")
                nc.tensor.matmul(out=ap[:],
                                 lhsT=agts_cm[:, k * 128:(k + 1) * 128],
                                 rhs=cs["awT"][:], start=True, stop=False)
                nc.tensor.matmul(out=ap[:], lhsT=s_tab[:, k, :],
                                 rhs=cs["xw2T"][:], start=False, stop=True)
                r3, b3 = gn_small(ap, "p3a")
                a1 = ep.tile([128, 128], BF16, tag="a1", name=f"a1_{k}")
                nc.scalar.activation(out=a1[:], in_=ap[:], func=AF.Relu,
                                     bias=b3[:], scale=r3[:])
                a1c = ep.tile([128, 128], BF16, tag="a1c", bufs=3,
                              name=f"a1c{k}")
                nc.scalar.dma_start(out=a1c[:], in_=a1[:], transpose=True)
                ep_pending.append((k, a1c))

            def epilogue_tail():
                k, a1c = ep_pending.pop(0)
                a2 = pu.tile([128, 128], F32, tag="u", name=f"a2_{k}")
                nc.tensor.matmul(out=a2[:], lhsT=a1c[:], rhs=cs["lwT"][:],
                                 start=True, stop=True)
                r4, b4 = gn_small(a2, "p3b")
                a2n = ep.tile([128, 128], F32, tag="a2n", name=f"a2n{k}")
                nc.scalar.activation(out=a2n[:], in_=a2[:], func=AF.Identity,
                                     bias=b4[:], scale=r4[:])
                osb = ep.tile([128, 128], F32, tag="osb", name=f"osb{k}")
                nc.vector.tensor_tensor(
                    out=osb[:], in0=a2n[:],
                    in1=agts_nm[:, k * D:(k + 1) * D], op=OP.add)
                o2 = ep.tile([128, 128], F32, tag="o2", name=f"o2_{k}")
                nc.vector.tensor_scalar_max(out=o2[:], in0=osb[:], scalar1=0.0)
                nc.gpsimd.dma_start(out=out_d[k * 128:(k + 1) * 128, :],
                                    in_=o2[:])

            # ---- phase 2: edge pipeline, software-pipelined one super deep
            # (super s's u-stage runs while s+1's d-stage fills the queues,
            # hiding the dcm4 DMA-transpose completion latency) ----
            win_ps = {}

            def u_stage(s, ctxg_t, dcm4, msb4, m2_4):
                for half in range(2):
                    ust = sm.tile([128, 12], F32, tag="ust",
                                  name=f"ust{s}_{half}")
                    umv = sm.tile([128, 4], F32, tag="umv",
                                  name=f"umv{s}_{half}")
                    us = []
                    for j in (0, 1):
                        i = half * 2 + j
                        gi = s * 4 + i
                        k = int(tile_slot[gi])
                        u = pu.tile([128, 128], F32, tag="u",
                                    name=f"u{s}_{i}")
                        nc.tensor.matmul(
                            out=u[:], lhsT=ctxg_t[:, i * 128:(i + 1) * 128],
                            rhs=cs["CT"][:], start=True, stop=False)
                        nc.tensor.matmul(
                            out=u[:], lhsT=m2_4[:, i, :],
                            rhs=qb_tab[:, k, :], start=False, stop=False)
                        nc.tensor.matmul(out=u[:], lhsT=dcm4[:, i, :],
                                         rhs=cs["AT"][:], start=False,
                                         stop=True)
                        nc.vector.bn_stats(out=ust[:, 6 * j:6 * j + 6],
                                           in_=u[:])
                        nc.vector.bn_aggr(out=umv[:, 2 * j:2 * j + 2],
                                          in_=ust[:, 6 * j:6 * j + 6])
                        us.append(u)
                    usd = sm.tile([128, 2], F32, tag="usd",
                                  name=f"usd{s}_{half}")
                    nc.scalar.activation(out=usd[:], in_=umv[:, 1::2],
                                         func=AF.Sqrt, bias=eps_t[:],
                                         scale=1.0)
                    ur = sm.tile([128, 2], F32, tag="ur", name=f"ur{s}_{half}")
                    nc.vector.reciprocal(out=ur[:], in_=usd[:])
                    ub = sm.tile([128, 2], F32, tag="ub", name=f"ub{s}_{half}")
                    nc.vector.scalar_tensor_tensor(
                        out=ub[:], in0=umv[:, 0::2], scalar=-1.0, in1=ur[:],
                        op0=OP.mult, op1=OP.mult)

                    for j in (0, 1):
                        i = half * 2 + j
                        gi = s * 4 + i
                        k = int(tile_slot[gi])
                        c_sb = wkp.tile([128, 128], BF16, tag="csb", bufs=6,
                                        name=f"csb{s}_{i}")
                        nc.scalar.activation(
                            out=c_sb[:], in_=us[j][:], func=AF.Relu,
                            bias=ub[:, j:j + 1], scale=ur[:, j:j + 1])
                        if gi == first_tile[k]:
                            win_ps[k] = pwin.tile([128, 128], F32, tag="win",
                                                  name=f"win{k}")
                        nc.tensor.matmul(
                            out=win_ps[k][:], lhsT=c_sb[:],
                            rhs=msb4[:, i * 128:(i + 1) * 128],
                            start=(gi == first_tile[k]),
                            stop=(gi == last_tile[k]))
                        if gi == last_tile[k]:
                            nc.vector.tensor_copy(out=s_tab[:, k, :],
                                                  in_=win_ps[k][:])
                            del win_ps[k]
                            if ep_pending:
                                epilogue_tail()
                            epilogue_head(k)

            pend = None
            for s in range(n_super):
                sl = slice(s * 512, (s + 1) * 512)
                dctr_t = io.tile([3, 512], BF16, tag="dctr", bufs=4,
                                 name=f"dctr{s}")
                nc.sync.dma_start(out=dctr_t[:], in_=din["dctr"][:, sl])
                big_t = io.tile([128, 512], BF16, tag="big", bufs=4,
                                name=f"big{s}")
                nc.sync.dma_start(out=big_t[:], in_=din["big"][:, sl])

                # masks: one-hot built from seg on gpsimd, node-major twin
                # via the batched XBAR transpose (prefetchable, const deps)
                msb4 = wkp.tile([128, 512], BF16, tag="msb4", bufs=3,
                                name=f"msb4_{s}")
                for i in range(4):
                    gi = s * 4 + i
                    nc.vector.tensor_scalar(
                        out=msb4[:, i * 128:(i + 1) * 128], in0=iota_t[:],
                        scalar1=seg_sb[:, gi:gi + 1], scalar2=None,
                        op0=OP.is_equal)
                m2_4 = wkp.tile([128, 4, 128], BF16, tag="m24", bufs=3,
                                name=f"m24_{s}")
                nc.scalar.dma_start(out=m2_4[:], in_=msb4[:], transpose=True)

                hp = ph.tile([128, 512], F32, tag="hp", name=f"hp{s}")
                nc.tensor.matmul(out=hp[:], lhsT=cs["w1aT"][:], rhs=dctr_t[:],
                                 start=True, stop=True)
                h_sb = wkp.tile([128, 512], BF16, tag="h", bufs=3,
                                name=f"h{s}")
                nc.scalar.activation(out=h_sb[:], in_=hp[:], func=AF.Relu,
                                     bias=0.0, scale=1.0)

                d4 = wkp.tile([128, 512], BF16, tag="d4", bufs=3,
                              name=f"d4_{s}")
                for half in range(2):
                    dst = sm.tile([128, 12], F32, tag="dst",
                                  name=f"dst{s}_{half}")
                    dmv = sm.tile([128, 4], F32, tag="dmv",
                                  name=f"dmv{s}_{half}")
                    dps = []
                    for j in (0, 1):
                        i = half * 2 + j
                        dp = pdp.tile([128, 128], F32, tag="dp",
                                      name=f"dp{s}_{i}")
                        nc.tensor.matmul(
                            out=dp[:], lhsT=h_sb[:, i * 128:(i + 1) * 128],
                            rhs=cs["w2T"][:], start=True, stop=True)
                        nc.vector.bn_stats(out=dst[:, 6 * j:6 * j + 6],
                                           in_=dp[:])
                        nc.vector.bn_aggr(out=dmv[:, 2 * j:2 * j + 2],
                                          in_=dst[:, 6 * j:6 * j + 6])
                        dps.append(dp)
                    dsd = sm.tile([128, 2], F32, tag="dsd",
                                  name=f"dsd{s}_{half}")
                    nc.scalar.activation(out=dsd[:], in_=dmv[:, 1::2],
                                         func=AF.Sqrt, bias=eps_t[:],
                                         scale=1.0)
                    dr = sm.tile([128, 2], F32, tag="dr", name=f"dr{s}_{half}")
                    nc.vector.reciprocal(out=dr[:], in_=dsd[:])
                    db = sm.tile([128, 2], F32, tag="db", name=f"db{s}_{half}")
                    nc.vector.scalar_tensor_tensor(
                        out=db[:], in0=dmv[:, 0::2], scalar=-1.0, in1=dr[:],
                        op0=OP.mult, op1=OP.mult)
                    for j in (0, 1):
                        i = half * 2 + j
                        nc.scalar.activation(
                            out=d4[:, i * 128:(i + 1) * 128], in_=dps[j][:],
                            func=AF.Relu, bias=db[:, j:j + 1],
                            scale=dr[:, j:j + 1])

                dcm4 = wkp.tile([128, 4, 128], BF16, tag="dcm4", bufs=3,
                                name=f"dcm4_{s}")
                nc.scalar.dma_start(out=dcm4[:], in_=d4[:], transpose=True)

                if pend is not None:
                    u_stage(*pend)
                pend = (s, big_t, dcm4, msb4, m2_4)
            u_stage(*pend)
            while ep_pending:
                epilogue_tail()
    _split_excess_waits(nc)
    return nc


def kernel(**inputs):
    _apply_drain_patch()
    per_core, shared, meta = _prep(inputs)
    nc = _build(meta)
    in_maps = [{**per_core[c], **shared} for c in range(NC)]
    res = run_bass_kernel_spmd(nc, in_maps, core_ids=list(range(NC)))
    out = np.zeros((N_NODES, D), np.float32)
    slot_win = meta["slot_win"]
    for c in range(NC):
        oc = np.asarray(res.results[c]["out"], np.float32)
        for k in range(NWIN):
            w = int(slot_win[c, k])
            if w < 0:
                continue
            base = 128 * w
            nv = min(128, N_NODES - base)
            out[base:base + nv] = oc[k * 128:k * 128 + nv]
    return out


# revision 24
# speedup vs baseline: 7.3628x; 7.3628x over previous
"""Trainium2 8-core kernel for the LaneGCN-style A2A message-passing block.

Strategy (v2, memory-regime):
  - Host: sort edges by destination (hi); partition the 157 global 128-node
    windows across 8 cores with greedy edge balancing; per core, order its 20
    window slots by descending edge count so the SPMD-shared per-slot padding
    (max across cores) stays tight. Pre-gather per edge: ctx row, dctr (+ones
    row so dist_b1 folds into the matmul), and BOTH one-hot incidence layouts
    (edge-major for scatter, node-major for gather) as bf16 DMA payloads.
  - Device per 128-edge tile (all operands bf16, PSUM fp32):
      h   = relu(W1a @ dctr3)                    (per super, ch-major, N=512)
      dp  = h_tile' @ W2ᵀ                        (edge-major)
      d   = relu(GN(dp))      via bn_stats/aggr + fused ACT scale/bias/relu
      d_cm = DMA-XBAR transpose of d             (no PE transpose, no copy)
      u   = ctxg@Cᵀ + onehotᵀ@qb_tab + d@Aᵀ      (3 accumulated MMs)
      c   = relu(GN(u))       same fused pattern
      s[win] += cᵀ @ onehot                      (scatter matmul into PSUM)
    GN row math (sqrt/recip/-m*r) is pair-batched across 2 tiles to amortize
    the fixed per-op engine overhead.
  - Node phases: qb_tab = (relu(GN(agts@q_wᵀ)))@Bᵀ per window (phase 1);
    epilogue GN->lin->GN->residual->relu per window, emitted inline at each
    window's last scatter so it overlaps the edge pipeline (phase 3).
"""

import sys

import numpy as np

if "/opt/trn_rl_repo" not in sys.path:
    sys.path.insert(0, "/opt/trn_rl_repo")

import ml_dtypes

import concourse.bass as bass
import concourse.mybir as mybir
import concourse.tile as tile
from concourse.bass_utils import run_bass_kernel_spmd

N_NODES = 20000
D = 128
NC = 8
NWIN = 20            # window slots per core (157 global windows over 8 cores)
NPAD = NWIN * 128
GWIN = (N_NODES + 127) // 128
F32 = mybir.dt.float32
BF16 = mybir.dt.bfloat16
NPBF = ml_dtypes.bfloat16


def _apply_drain_patch():
    """This neuronxcc build rejects >2 sem waits on the Tile tail drain
    ("Too many sync wait commands"); split them into single-sem SP waits."""
    from concourse.vector_clock import ScopedClock

    if getattr(tile.TileContext, "_drain_patched", False):
        return

    def _patched(self, tick_clock, wait_clock):
        nc = self.nc
        probe = nc.sync.nop(nofuse=True, hint="drain_wait_probe")
        wait_clock.add_sem_waits(
            probe.ins, ScopedClock({None: tick_clock.global_clock})
        )
        si = probe.ins.sync_info
        waits = list(si.on_wait) if si and si.on_wait else []
        sem_by_id = {h.num: h for h in self.sems.allocated().values()}
        if len(waits) > 2:
            si.on_wait.clear()
            for w in waits:
                h = sem_by_id[w.id]
                nc.sync.wait_ge(h, w.wait_value)
        nc.sync.drain()
        nc.all_engine_barrier()
        popped = nc._tile_sem_poison_stack.pop()
        assert popped is self._sem_poison
        nc.clear_and_free_semaphores(list(self.sems.allocated().values()))
        nc.all_engine_barrier()

    tile.TileContext._drain_and_barrier = _patched
    tile.TileContext._drain_patched = True


def _split_excess_waits(nc, max_waits=1):
    """walrus here rejects instructions with >2 sem-wait commands; hoist the
    excess onto single-wait NoOps inserted just before (same engine)."""
    n = 0
    for f in nc.m.functions:
        for bb in f.blocks:
            out = []
            changed = False
            for ins in bb.instructions:
                si = ins.sync_info
                waits = list(si.on_wait) if si and si.on_wait else []
                if len(waits) > max_waits:
                    keep = waits[-max_waits:]
                    for w in waits[:-max_waits]:
                        nop = mybir.InstNoOp(
                            name=f"I-waitfix-{n}", engine=ins.engine
                        )
                        n += 1
                        nop.sync_info = mybir.SyncInfo(
                            on_wait=[w], on_update=[]
                        )
                        out.append(nop)
                    ins.sync_info = mybir.SyncInfo(
                        on_wait=keep,
                        on_update=list(si.on_update) if si.on_update else [],
                    )
                    changed = True
                out.append(ins)
            if changed:
                bb.instructions = out


def _prep(inputs):
    """Sort/balance/pad edges; build per-core bf16 device arrays."""
    f = lambda k: np.asarray(inputs[k], dtype=np.float32)
    agts = f("agts")
    ctx = f("ctx")
    agt_ctrs = f("agt_ctrs")
    ctx_ctrs = f("ctx_ctrs")
    hi = np.asarray(inputs["hi"], dtype=np.int64)
    wi = np.asarray(inputs["wi"], dtype=np.int64)

    for g, b in (("dist_g", "dist_beta"), ("q_g", "q_beta"),
                 ("ctx_g", "ctx_beta"), ("norm_g", "norm_beta"),
                 ("lin_g", "lin_beta")):
        assert np.allclose(np.asarray(inputs[g]), 1.0), f"{g} != 1 unsupported"
        assert np.allclose(np.asarray(inputs[b]), 0.0), f"{b} != 0 unsupported"

    order = np.argsort(hi, kind="stable")
    hi_s = hi[order]
    wi_s = wi[order]

    starts = np.searchsorted(hi_s, np.arange(GWIN) * 128)
    ends = np.searchsorted(
        hi_s, np.minimum((np.arange(GWIN) + 1) * 128, N_NODES))
    cnt = ends - starts

    # greedy balance: biggest windows first to the least-loaded core
    core_load = np.zeros(NC, np.int64)
    core_wins = [[] for _ in range(NC)]
    for w in np.argsort(-cnt, kind="stable"):
        cands = [c for c in range(NC) if len(core_wins[c]) < NWIN]
        c = min(cands, key=lambda c: core_load[c])
        core_wins[c].append(w)
        core_load[c] += cnt[w]

    slot_win = np.full((NC, NWIN), -1, np.int64)
    for c in range(NC):
        ws = sorted(core_wins[c], key=lambda w: -cnt[w])
        slot_win[c, :len(ws)] = ws
    slot_cnt = np.where(slot_win >= 0, cnt[np.maximum(slot_win, 0)], 0)

    wk = ((slot_cnt.max(axis=0) + 127) // 128) * 128
    wk = np.maximum(wk, 128)
    e_pad = int(wk.sum())
    extra = (-e_pad) % 512
    wk[NWIN - 1] += extra
    e_pad += extra
    woff = np.concatenate([[0], np.cumsum(wk)]).astype(np.int64)
    n_tiles = e_pad // 128

    tile_slot = np.empty(n_tiles, np.int64)
    for k in range(NWIN):
        tile_slot[woff[k] // 128: woff[k + 1] // 128] = k
    first_tile = (woff[:-1] // 128).astype(np.int64)
    last_tile = (woff[1:] // 128 - 1).astype(np.int64)

    bf = lambda a: np.ascontiguousarray(a).astype(NPBF)
    w1 = f("dist_w1")                                          # [D, 2]
    b1 = f("dist_b1")                                          # [D]
    cw1 = f("ctx_w1")                                          # [D, 3D]
    w1a = np.zeros((128, 128), np.float32)
    w1a[0:2] = w1.T
    w1a[2] = b1
    shared_cols = bf(np.concatenate(
        [w1a, f("dist_w2").T, cw1[:, :D].T, cw1[:, D:2 * D].T,
         cw1[:, 2 * D:].T, f("q_w").T, f("ctx_w2").T, f("agt_w").T,
         f("lin_w").T], axis=1))                               # [128, 1152]

    per_core = []
    for c in range(NC):
        dctr3 = np.zeros((3, e_pad), np.float32)
        ctxg = np.zeros((e_pad, D), np.float32)
        seg = np.full(e_pad, -1, np.int64)
        nodes = np.full(NWIN * 128, -1, np.int64)
        for k in range(NWIN):
            w = int(slot_win[c, k])
            if w < 0:
                continue
            n = int(cnt[w])
            s0 = int(starts[w])
            d0 = int(woff[k])
            idx = slice(s0, s0 + n)
            dctr3[0, d0:d0 + n] = agt_ctrs[hi_s[idx], 0] - ctx_ctrs[wi_s[idx], 0]
            dctr3[1, d0:d0 + n] = agt_ctrs[hi_s[idx], 1] - ctx_ctrs[wi_s[idx], 1]
            dctr3[2, d0:d0 + n] = 1.0
            ctxg[d0:d0 + n] = ctx[wi_s[idx]]
            seg[d0:d0 + n] = hi_s[idx] - 128 * w
            base = 128 * w
            nv = min(128, N_NODES - base)
            nodes[k * 128:k * 128 + nv] = np.arange(base, base + nv)

        # per-tile seg values, edge-major [128, n_tiles]: column t holds the
        # window-local destination offset of each edge in tile t (-1 pad)
        seg_pm = np.ascontiguousarray(
            seg.reshape(n_tiles, 128).T).astype(np.float32)
        valid = nodes >= 0
        ag = agts[np.maximum(nodes, 0)] * valid[:, None]       # [NPAD, D]
        # head blob: [w1aT pad | weights 8x128 | agts_cm | agts_nm]
        head = np.concatenate(
            [shared_cols, np.ascontiguousarray(ag.T).astype(NPBF),
             np.ascontiguousarray(
                 ag.reshape(NWIN, 128, D).transpose(1, 0, 2)
                 .reshape(128, NWIN * D)).astype(NPBF)], axis=1)
        per_core.append(dict(
            dctr=np.ascontiguousarray(dctr3).astype(NPBF),
            big=np.ascontiguousarray(ctxg.T).astype(NPBF),     # [D, e_pad]
            head=np.ascontiguousarray(head),
            seg_pm=seg_pm,
        ))

    meta = dict(e_pad=e_pad, n_tiles=n_tiles, tile_slot=tile_slot,
                first_tile=first_tile, last_tile=last_tile, slot_win=slot_win)
    return per_core, {}, meta


def _build(meta):
    nc = bass.Bass()
    e_pad = meta["e_pad"]
    n_tiles = meta["n_tiles"]
    tile_slot = meta["tile_slot"]
    first_tile = meta["first_tile"]
    last_tile = meta["last_tile"]
    n_super = e_pad // 512
    AF = mybir.ActivationFunctionType
    OP = mybir.AluOpType

    din = {}
    for name, shape, dt in [
        ("dctr", [3, e_pad], BF16), ("big", [D, e_pad], BF16),
        ("head", [128, 1152 + 2 * NPAD], BF16),
        ("seg_pm", [128, n_tiles], F32),
    ]:
        din[name] = nc.dram_tensor(name, shape, dt, kind="ExternalInput")
    out_d = nc.dram_tensor("out", [NPAD, D], F32, kind="ExternalOutput")

    with tile.TileContext(nc) as tc:
        with (
            tc.tile_pool(name="consts", bufs=1) as consts,
            tc.tile_pool(name="io", bufs=3) as io,
            tc.tile_pool(name="wkp", bufs=4) as wkp,
            tc.tile_pool(name="ep", bufs=2) as ep,
            tc.tile_pool(name="sm", bufs=4) as sm,
            tc.tile_pool(name="smp", bufs=3) as smp,
            tc.tile_pool(name="ph", bufs=1, space="PSUM") as ph,
            tc.tile_pool(name="pdp", bufs=3, space="PSUM") as pdp,
            tc.tile_pool(name="pu", bufs=3, space="PSUM") as pu,
            tc.tile_pool(name="pwin", bufs=1, space="PSUM") as pwin,
        ):
            cs = {}
            names = ("w1aT", "w2T", "AT", "BT", "CT", "qwT", "xw2T",
                     "awT", "lwT")
            for idx, name in enumerate(names):
                shape = [3, D] if name == "w1aT" else [D, D]
                t = consts.tile(shape, BF16, tag=f"c_{name}", name=f"c_{name}")
                nc.sync.dma_start(
                    out=t[:],
                    in_=din["head"][0:shape[0], idx * 128:idx * 128 + 128])
                cs[name] = t
            agts_cm = consts.tile([D, NPAD], BF16, tag="c_agcm")
            nc.sync.dma_start(out=agts_cm[:],
                              in_=din["head"][:, 1152:1152 + NPAD])
            agts_nm = consts.tile([128, NWIN * D], BF16, tag="c_agnm")
            nc.sync.dma_start(out=agts_nm[:],
                              in_=din["head"][:, 1152 + NPAD:1152 + 2 * NPAD])
            seg_sb = consts.tile([128, n_tiles], F32, tag="c_seg")
            nc.sync.dma_start(out=seg_sb[:], in_=din["seg_pm"][:])
            qb_tab = consts.tile([128, NWIN, 128], BF16, tag="c_qbtab")
            s_tab = consts.tile([128, NWIN, 128], BF16, tag="c_stab")
            eps_t = consts.tile([128, 1], F32, tag="c_eps")
            nc.vector.memset(eps_t[:], 1e-5)
            iota_t = consts.tile([128, 128], F32, tag="c_iota")
            nc.gpsimd.iota(iota_t[:], pattern=[[1, 128]], base=0,
                           channel_multiplier=0,
                           allow_small_or_imprecise_dtypes=True)

            def gn_small(ps, tagp):
                """Unbatched GN row stats for node phases: returns (r, b)."""
                st = smp.tile([128, 6], F32, tag=f"{tagp}st", name=f"{tagp}st")
                nc.vector.bn_stats(out=st[:], in_=ps[:])
                mv = smp.tile([128, 2], F32, tag=f"{tagp}mv", name=f"{tagp}mv")
                nc.vector.bn_aggr(out=mv[:], in_=st[:])
                sd = smp.tile([128, 1], F32, tag=f"{tagp}sd", name=f"{tagp}sd")
                nc.scalar.activation(out=sd[:], in_=mv[:, 1:2], func=AF.Sqrt,
                                     bias=eps_t[:], scale=1.0)
                r = smp.tile([128, 1], F32, tag=f"{tagp}r", name=f"{tagp}r")
                nc.vector.reciprocal(out=r[:], in_=sd[:])
                b = smp.tile([128, 1], F32, tag=f"{tagp}b", name=f"{tagp}b")
                nc.vector.tensor_scalar(
                    out=b[:], in0=mv[:, 0:1], scalar1=r[:], scalar2=-1.0,
                    op0=OP.mult, op1=OP.mult,
                )
                return r, b

            # ---- phase 1: qb table (q -> GN -> relu -> @B^T per window) ----
            for w in range(NWIN):
                qp = pdp.tile([128, 128], F32, tag="dp", name=f"qp{w}")
                nc.tensor.matmul(out=qp[:],
                                 lhsT=agts_cm[:, w * 128:(w + 1) * 128],
                                 rhs=cs["qwT"][:], start=True, stop=True)
                r1, b1 = gn_small(qp, "p1")
                q_sb = ep.tile([128, 128], BF16, tag="qsb", name=f"qsb{w}")
                nc.scalar.activation(out=q_sb[:], in_=qp[:], func=AF.Relu,
                                     bias=b1[:], scale=r1[:])
                q_cm = ep.tile([128, 128], BF16, tag="qcm", name=f"qcm{w}")
                nc.scalar.dma_start(out=q_cm[:], in_=q_sb[:], transpose=True)
                qbp = pu.tile([128, 128], F32, tag="u", name=f"qbp{w}")
                nc.tensor.matmul(out=qbp[:], lhsT=q_cm[:], rhs=cs["BT"][:],
                                 start=True, stop=True)
                nc.vector.tensor_copy(out=qb_tab[:, w, :], in_=qbp[:])

            # ---- phase 3, two stages so the a1 transpose latency is hidden
            # behind a few supers of edge work before the tail runs ----
            ep_pending = []

            def epilogue_head(k):
                ap = pdp.tile([128, 128], F32, tag="dp", name=f"ap{k}")
                nc.tensor.matmul(out=ap[:],
                                 lhsT=agts_cm[:, k * 128:(k + 1) * 128],
                                 rhs=cs["awT"][:], start=True, stop=False)
                nc.tensor.matmul(out=ap[:], lhsT=s_tab[:, k, :],
                                 rhs=cs["xw2T"][:], start=False, stop=True)
                r3, b3 = gn_small(ap, "p3a")
                a1 = ep.tile([128, 128], BF16, tag="a1", name=f"a1_{k}")
                nc.scalar.activation(out=a1[:], in_=ap[:], func=AF.Relu,
                                     bias=b3[:], scale=r3[:])
                a1c = ep.tile([128, 128], BF16, tag="a1c", bufs=3,
                              name=f"a1c{k}")
                nc.scalar.dma_start(out=a1c[:], in_=a1[:], transpose=True)
                ep_pending.append((k, a1c))

            def epilogue_tail():
                k, a1c = ep_pending.pop(0)
                a2 = pu.tile([128, 128], F32, tag="u", name=f"a2_{k}")
                nc.tensor.matmul(out=a2[:], lhsT=a1c[:], rhs=cs["lwT"][:],
                                 start=True, stop=True)
                r4, b4 = gn_small(a2, "p3b")
                a2n = ep.tile([128, 128], F32, tag="a2n", name=f"a2n{k}")
                nc.scalar.activation(out=a2n[:], in_=a2[:], func=AF.Identity,
                                     bias=b4[:], scale=r4[:])
                osb = ep.tile([128, 128], F32, tag="osb", name=f"osb{k}")
                nc.vector.tensor_tensor(
                    out=osb[:], in0=a2n[:],
                    in1=agts_nm[:, k * D:(k + 1) * D], op=OP.add)
                o2 = ep.tile([128, 128], F32, tag="o2", name=f"o2_{k}")
                nc.vector.tensor_scalar_max(out=o2[:], in0=osb[:], scalar1=0.0)
                nc.gpsimd.dma_start(out=out_d[k * 128:(k + 1) * 128, :],
                                    in_=o2[:])

            # ---- phase 2: edge pipeline, software-pipelined one super deep
            # (super s's u-stage runs while s+1's d-stage fills the queues,
            # hiding the dcm4 DMA-transpose completion latency) ----
            win_ps = {}

            def u_stage(s, ctxg_t, dcm4, msb4, m2_4):
                for half in range(2):
                    ust = sm.tile([128, 12], F32, tag="ust",
                                  name=f"ust{s}_{half}")
                    umv = sm.tile([128, 4], F32, tag="umv",
                                  name=f"umv{s}_{half}")
                    us = []
                    for j in (0, 1):
                        i = half * 2 + j
                        gi = s * 4 + i
                        k = int(tile_slot[gi])
                        u = pu.tile([128, 128], F32, tag="u",
                                    name=f"u{s}_{i}")
                        nc.tensor.matmul(
                            out=u[:], lhsT=ctxg_t[:, i * 128:(i + 1) * 128],
                            rhs=cs["CT"][:], start=True, stop=False)
                        nc.tensor.matmul(
                            out=u[:], lhsT=m2_4[:, i, :],
                            rhs=qb_tab[:, k, :], start=False, stop=False)
                        nc.tensor.matmul(out=u[:], lhsT=dcm4[:, i, :],
                                         rhs=cs["AT"][:], start=False,
                                         stop=True)
                        nc.vector.bn_stats(out=ust[:, 6 * j:6 * j + 6],
                                           in_=u[:])
                        nc.vector.bn_aggr(out=umv[:, 2 * j:2 * j + 2],
                                          in_=ust[:, 6 * j:6 * j + 6])
                        us.append(u)
                    usd = sm.tile([128, 2], F32, tag="usd",
                                  name=f"usd{s}_{half}")
                    nc.scalar.activation(out=usd[:], in_=umv[:, 1::2],
                                         func=AF.Sqrt, bias=eps_t[:],
                                         scale=1.0)
                    ur = sm.tile([128, 2], F32, tag="ur", name=f"ur{s}_{half}")
                    nc.vector.reciprocal(out=ur[:], in_=usd[:])
                    ub = sm.tile([128, 2], F32, tag="ub", name=f"ub{s}_{half}")
                    nc.vector.scalar_tensor_tensor(
                        out=ub[:], in0=umv[:, 0::2], scalar=-1.0, in1=ur[:],
                        op0=OP.mult, op1=OP.mult)

                    for j in (0, 1):
                        i = half * 2 + j
                        gi = s * 4 + i
                        k = int(tile_slot[gi])
                        c_sb = wkp.tile([128, 128], BF16, tag="csb", bufs=6,
                                        name=f"csb{s}_{i}")
                        nc.scalar.activation(
                            out=c_sb[:], in_=us[j][:], func=AF.Relu,
                            bias=ub[:, j:j + 1], scale=ur[:, j:j + 1])
                        if gi == first_tile[k]:
                            win_ps[k] = pwin.tile([128, 128], F32, tag="win",
                                                  name=f"win{k}")
                        nc.tensor.matmul(
                            out=win_ps[k][:], lhsT=c_sb[:],
                            rhs=msb4[:, i * 128:(i + 1) * 128],
                            start=(gi == first_tile[k]),
                            stop=(gi == last_tile[k]))
                        if gi == last_tile[k]:
                            nc.vector.tensor_copy(out=s_tab[:, k, :],
                                                  in_=win_ps[k][:])
                            del win_ps[k]
                            if ep_pending:
                                epilogue_tail()
                            epilogue_head(k)

            pend = []
            for s in range(n_super):
                sl = slice(s * 512, (s + 1) * 512)
                dctr_t = io.tile([3, 512], BF16, tag="dctr", bufs=6,
                                 name=f"dctr{s}")
                nc.sync.dma_start(out=dctr_t[:], in_=din["dctr"][:, sl])
                big_t = io.tile([128, 512], BF16, tag="big", bufs=6,
                                name=f"big{s}")
                nc.sync.dma_start(out=big_t[:], in_=din["big"][:, sl])

                # masks: one-hot built from seg on gpsimd, node-major twin
                # via the batched XBAR transpose (prefetchable, const deps)
                msb4 = wkp.tile([128, 512], BF16, tag="msb4", bufs=5,
                                name=f"msb4_{s}")
                for i in range(4):
                    gi = s * 4 + i
                    nc.vector.tensor_scalar(
                        out=msb4[:, i * 128:(i + 1) * 128], in0=iota_t[:],
                        scalar1=seg_sb[:, gi:gi + 1], scalar2=None,
                        op0=OP.is_equal)
                m2_4 = wkp.tile([128, 4, 128], BF16, tag="m24", bufs=5,
                                name=f"m24_{s}")
                nc.scalar.dma_start(out=m2_4[:], in_=msb4[:], transpose=True)

                hp = ph.tile([128, 512], F32, tag="hp", name=f"hp{s}")
                nc.tensor.matmul(out=hp[:], lhsT=cs["w1aT"][:], rhs=dctr_t[:],
                                 start=True, stop=True)
                h_sb = wkp.tile([128, 512], BF16, tag="h", bufs=4,
                                name=f"h{s}")
                nc.scalar.activation(out=h_sb[:], in_=hp[:], func=AF.Relu,
                                     bias=0.0, scale=1.0)

                d4 = wkp.tile([128, 512], BF16, tag="d4", bufs=4,
                              name=f"d4_{s}")
                for half in range(2):
                    dst = sm.tile([128, 12], F32, tag="dst",
                                  name=f"dst{s}_{half}")
                    dmv = sm.tile([128, 4], F32, tag="dmv",
                                  name=f"dmv{s}_{half}")
                    dps = []
                    for j in (0, 1):
                        i = half * 2 + j
                        dp = pdp.tile([128, 128], F32, tag="dp",
                                      name=f"dp{s}_{i}")
                        nc.tensor.matmul(
                            out=dp[:], lhsT=h_sb[:, i * 128:(i + 1) * 128],
                            rhs=cs["w2T"][:], start=True, stop=True)
                        nc.vector.bn_stats(out=dst[:, 6 * j:6 * j + 6],
                                           in_=dp[:])
                        nc.vector.bn_aggr(out=dmv[:, 2 * j:2 * j + 2],
                                          in_=dst[:, 6 * j:6 * j + 6])
                        dps.append(dp)
                    dsd = sm.tile([128, 2], F32, tag="dsd",
                                  name=f"dsd{s}_{half}")
                    nc.scalar.activation(out=dsd[:], in_=dmv[:, 1::2],
                                         func=AF.Sqrt, bias=eps_t[:],
                                         scale=1.0)
                    dr = sm.tile([128, 2], F32, tag="dr", name=f"dr{s}_{half}")
                    nc.vector.reciprocal(out=dr[:], in_=dsd[:])
                    db = sm.tile([128, 2], F32, tag="db", name=f"db{s}_{half}")
                    nc.vector.scalar_tensor_tensor(
                        out=db[:], in0=dmv[:, 0::2], scalar=-1.0, in1=dr[:],
                        op0=OP.mult, op1=OP.mult)
                    for j in (0, 1):
                        i = half * 2 + j
                        nc.scalar.activation(
                            out=d4[:, i * 128:(i + 1) * 128], in_=dps[j][:],
                            func=AF.Relu, bias=db[:, j:j + 1],
                            scale=dr[:, j:j + 1])

                dcm4 = wkp.tile([128, 4, 128], BF16, tag="dcm4", bufs=5,
                                name=f"dcm4_{s}")
                nc.scalar.dma_start(out=dcm4[:], in_=d4[:], transpose=True)

                pend.append((s, big_t, dcm4, msb4, m2_4))
                if len(pend) > 2:
                    u_stage(*pend.pop(0))
            while pend:
                u_stage(*pend.pop(0))
            while ep_pending:
                epilogue_tail()
    _split_excess_waits(nc)
    return nc


def kernel(**inputs):
    _apply_drain_patch()
    per_core, shared, meta = _prep(inputs)
    nc = _build(meta)
    in_maps = [{**per_core[c], **shared} for c in range(NC)]
    res = run_bass_kernel_spmd(nc, in_maps, core_ids=list(range(NC)))
    out = np.zeros((N_NODES, D), np.float32)
    slot_win = meta["slot_win"]
    for c in range(NC):
        oc = np.asarray(res.results[c]["out"], np.float32)
        for k in range(NWIN):
            w = int(slot_win[c, k])
            if w < 0:
                continue
            base = 128 * w
            nv = min(128, N_NODES - base)
            out[base:base + nv] = oc[k * 128:k * 128 + nv]
    return out
